# revision 1
# baseline (speedup 1.0000x reference)
"""Trainium2 Bass kernel for the MAVE global-epistasis measurement layer.

    y[b] = a_0 + sum_k bk[k] * tanh( (ck @ z[b])[k] + dk[k] )
    z: [2097152, 16] f32, ck: [64, 16], bk, dk: [64], a_0: [1]

Data-parallel over 8 NeuronCores (262144 batch rows per core).

Per-core dataflow (Tile kernel):
  - z loaded in [128, 512] tiles, partition p holding 32 consecutive batch
    rows enumerated (u, jb) = (16-row half, row within half) so a later DVE
    32x32 block transpose yields z^T blocks.
  - DVE stream-transpose -> z^T layout [32a + 16v + z, 32B + c] =
    z[base + 1024a + 32c + 16v + B, z], rounded to float32r on GpSimd.
  - TensorE: block-diagonal ck weights ckbd_a [128,128] (f32r, 1 cyc/row)
    produce pre-h [(v,k), col] tiles; ScalarE applies tanh(x + dk) into bf16.
  - TensorE: bk block weights [128, 32] reduce over k into y PSUM [32, 512],
    16 (slot) groups accumulated per span.
  - DVE evacuates y (+a_0) with a column permutation so the final HWDGE DMA
    writes contiguous 64-byte runs to DRAM.
"""
import numpy as np

import concourse.bass as bass
import concourse.tile as tile
from concourse import mybir
from concourse.bass_utils import run_bass_kernel_spmd

from contextlib import ExitStack

F32 = mybir.dt.float32
F32R = mybir.dt.float32r
BF16 = mybir.dt.bfloat16
U32 = mybir.dt.uint32

B_FULL = 2097152
N_CORES = 8
NC_ROWS = B_FULL // N_CORES          # 262144
SUPER = 4096                         # rows per transpose tile
N_SUPER = NC_ROWS // SUPER           # 64
N_GROUPS = N_SUPER * 4               # 256  (1024 rows each)
MEGA_ROWS = 65536                    # rows per output flush
N_MEGA = NC_ROWS // MEGA_ROWS        # 4
HTILE = 3                            # matmul groups per ACT tanh op


def _multiwait_split(nc):
    ctr = 0
    for f in nc.m.functions:
        for blk in f.blocks:
            insts = blk.instructions
            i = 0
            while i < len(insts):
                inst = insts[i]
                si = getattr(inst, "sync_info", None)
                if si is not None and si.on_wait and len(si.on_wait) > 1:
                    extra = list(si.on_wait[:-1])
                    del si.on_wait[:-1]
                    for w in extra:
                        ctr += 1
                        nop = mybir.InstNoOp(name=f"I-mws-{ctr}", ins=[], outs=[])
                        nop.engine = inst.engine
                        nop.sync_info = mybir.SyncInfo(on_wait=[w], on_update=[])
                        insts.insert(i, nop)
                        i += 1
                i += 1
    return nc


def build_nc():
    nc = bass.Bass()
    z_ext = nc.declare_dram_parameter("z", [NC_ROWS, 16], F32, isOutput=False)
    a0_ext = nc.declare_dram_parameter("a_0", [1], F32, isOutput=False)
    bk_ext = nc.declare_dram_parameter("bk", [64], F32, isOutput=False)
    ck_ext = nc.declare_dram_parameter("ck", [64, 16], F32, isOutput=False)
    dk_ext = nc.declare_dram_parameter("dk", [64], F32, isOutput=False)
    y_ext = nc.declare_dram_parameter("y", [NC_ROWS, 1], F32, isOutput=True)

    ctx = ExitStack()
    with ctx:
        tc = ctx.enter_context(tile.TileContext(nc))
        consts = ctx.enter_context(tc.tile_pool(name="consts", bufs=1))
        zn_pool = ctx.enter_context(tc.tile_pool(name="zn", bufs=3))
        zt_pool = ctx.enter_context(tc.tile_pool(name="zt", bufs=2))
        zr_pool = ctx.enter_context(tc.tile_pool(name="zr", bufs=2))
        hsb_pool = ctx.enter_context(tc.tile_pool(name="hsb", bufs=2))
        ysb_pool = ctx.enter_context(tc.tile_pool(name="ysb", bufs=2))
        hps_pool = ctx.enter_context(tc.tile_pool(name="hps", bufs=2, space="PSUM"))
        yps_pool = ctx.enter_context(tc.tile_pool(name="yps", bufs=1, space="PSUM"))

        # ---- constants -------------------------------------------------
        ckT = ck_ext[:].rearrange("k z -> z k")          # [16, 64] strided view
        ckbd = []
        for a in range(4):
            t = consts.tile([128, 128], F32R, tag=f"ckbd{a}")
            nc.vector.memset(t.bitcast(U32), 0)
            for v in range(2):
                nc.gpsimd.dma_start(
                    out=t[32 * a + 16 * v: 32 * a + 16 * v + 16,
                          64 * v: 64 * v + 64],
                    in_=ckT,
                )
            ckbd.append(t)

        bw = []
        for slot in range(16):
            t = consts.tile([128, 32], BF16, tag=f"bw{slot}")
            nc.vector.memset(t.bitcast(mybir.dt.uint16), 0)
            for v in range(2):
                m = 2 * slot + v
                nc.gpsimd.dma_start(
                    out=t[64 * v: 64 * v + 64, m: m + 1],
                    in_=bk_ext[:],
                )
            bw.append(t)

        dk_col = consts.tile([128, 1], F32, tag="dkcol")
        for v in range(2):
            nc.gpsimd.dma_start(out=dk_col[64 * v: 64 * v + 64, :], in_=dk_ext[:])
        a0_col = consts.tile([32, 1], F32, tag="a0col")
        nc.gpsimd.dma_start(out=a0_col, in_=a0_ext[:].to_broadcast((32, 1)))

        # DRAM views ------------------------------------------------------
        # z rows: b = s*4096 + 32p + 16u + jb ; z cols: z
        zd = z_ext[:].rearrange(
            "(s p u jb) z -> s p u jb z", s=N_SUPER, p=128, u=2, jb=16
        )
        # y: b = m*65536 + q*16384 + sl*1024 + c*32 + u*16 + jb
        yd = y_ext[:].rearrange(
            "(m q sl c u jb) one -> m q u sl c (jb one)",
            m=N_MEGA, q=4, sl=16, c=32, u=2, jb=16,
        )

        # ---- main loop ---------------------------------------------------
        h_ps = h_sb = y_ps = y_sb = None
        pend = []

        def flush_act():
            nonlocal pend, y_ps, y_sb
            if not pend:
                return
            ncols = len(pend) * 512
            nc.scalar.activation(
                h_sb[:, :ncols], h_ps[:, :ncols],
                mybir.ActivationFunctionType.Tanh,
                bias=dk_col, scale=1.0,
            )
            for gg, col in pend:
                slot = gg % 16
                if slot == 0:
                    y_ps = yps_pool.tile([32, 512], F32)
                nc.tensor.matmul(
                    y_ps, bw[slot], h_sb[:, col:col + 512],
                    start=(slot == 0), stop=(slot == 15),
                )
                if slot == 15:
                    q = (gg // 16) % 4
                    if q == 0:
                        y_sb = ysb_pool.tile([32, 2048], F32)
                    out_ap = y_sb[:, q * 512:(q + 1) * 512].rearrange(
                        "p (c jb) -> p jb c", c=32, jb=16)
                    in_ap = y_ps.rearrange("p (jb c) -> p jb c", jb=16, c=32)
                    nc.vector.tensor_scalar_add(out_ap, in_ap, a0_col)
                    if q == 3:
                        mega = gg // 64
                        ysrc = y_sb.rearrange(
                            "(sl v) (qq c jb) -> v sl qq c jb",
                            v=2, qq=4, c=32, jb=16)
                        for u in range(2):
                            for qq in range(4):
                                nc.sync.dma_start(
                                    out=yd[mega, qq, u],
                                    in_=ysrc[u, :, qq],
                                )
            pend = []

        for g in range(N_GROUPS):
            s, a = divmod(g, 4)
            if a == 0:
                zn = zn_pool.tile([128, 512], F32)
                zn4 = zn.rearrange("p (jb u z) -> p u jb z", jb=16, u=2, z=16)
                for u in range(2):
                    nc.sync.dma_start(out=zn4[:, u], in_=zd[s, :, u])
                zt32 = zt_pool.tile([128, 512], F32)
                nc.vector.transpose(zt32, zn)
                ztr = zr_pool.tile([128, 512], F32R)
                nc.gpsimd.tensor_copy(ztr, zt32)
            if g % HTILE == 0:
                h_ps = hps_pool.tile([128, HTILE * 512], F32)
                h_sb = hsb_pool.tile([128, HTILE * 512], BF16)
            col = (g % HTILE) * 512
            nc.tensor.matmul(
                h_ps[:, col:col + 512], ckbd[a], ztr, start=True, stop=True
            )
            pend.append((g, col))
            if g % HTILE == HTILE - 1 or g == N_GROUPS - 1:
                flush_act()

    _multiwait_split(nc)
    return nc


_NC_CACHE = None


def _get_nc():
    global _NC_CACHE
    if _NC_CACHE is None:
        _NC_CACHE = build_nc()
    return _NC_CACHE


def _run(inputs, **run_kwargs):
    nc = _get_nc()
    z = np.ascontiguousarray(np.asarray(inputs["z"], dtype=np.float32))
    a0 = np.asarray(inputs["a_0"], dtype=np.float32).reshape(1)
    bk = np.asarray(inputs["bk"], dtype=np.float32).reshape(64)
    ck = np.ascontiguousarray(np.asarray(inputs["ck"], dtype=np.float32))
    dk = np.asarray(inputs["dk"], dtype=np.float32).reshape(64)
    in_maps = []
    for c in range(N_CORES):
        in_maps.append({
            "z": z[c * NC_ROWS:(c + 1) * NC_ROWS],
            "a_0": a0, "bk": bk, "ck": ck, "dk": dk,
        })
    res = run_bass_kernel_spmd(nc, in_maps, core_ids=list(range(N_CORES)),
                               **run_kwargs)
    y = np.concatenate([res.results[c]["y"] for c in range(N_CORES)], axis=0)
    return y, res


def kernel(**inputs) -> np.ndarray:
    y, _ = _run(inputs)
    return y

